# revision 1
# baseline (speedup 1.0000x reference)
"""Trainium2 Bass kernel for nn_Blobber (3x3 box conv + steep sigmoid, x2).

The reference iterates 4 times but re-convolves the ORIGINAL input each
iteration, so all iterations are identical: the computation collapses to
    y = sigmoid((box3x3(sigmoid((box3x3(x) - 0.01*9) * 1000/9)) - 0.9*9) * 1000/9)
i.e. conv -> sigmoid -> conv -> sigmoid, once.

Implementation (per core, pure data-parallel over batch):
  Each separable 3-tap pass is a TensorE matmul with the image chunk as the
  stationary operand and a narrow banded (tridiagonal) matrix as the moving
  operand.  out[m,n] = sum_k lhsT[k,m] rhs[k,n] contracts the partition dim
  and transposes the layout, so alternating stages apply the vertical /
  horizontal passes with no explicit transposes and no halo exchange; the
  2-column band overlaps between contraction chunks accumulate in PSUM via
  the per-element has_written bits (first chunk start=True, rest accumulate).

  Dataflow per image (intermediates bf16, PSUM f32):
    SWDGE DMA-cast f32->bf16 -> [A] 16 MMs -> PSUM -> DVE copy -> bf16
    -> [B] 16 MMs -> PSUM -> ACT sigmoid(scale*x+bias) -> bf16
    -> [C] -> copy -> [D] -> ACT sigmoid -> f32 -> HWDGE store.
  Two-image lockstep emission keeps the PE busy while DVE/ACT drain PSUM,
  and ~30 scratch matmuls at the start warm the PE HAM clock gate to
  2.4 GHz while the first input DMA streams.

  bf16 is safe here: every sigmoid argument is saturated by >= ~50 (the
  output is exactly 0/1 everywhere), verified against the f32 reference.
"""

import sys

for _p in ("/opt/trn_rl_repo",):
    if _p not in sys.path:
        sys.path.append(_p)

import numpy as np
import ml_dtypes

import concourse.bass as bass
import concourse.mybir as mybir
from concourse import bacc
from concourse.tile import TileContext
from concourse.bass_utils import run_bass_kernel_spmd

N_CORES = 8
B = 32
H = W = 512
P = 128
NT = H // P                # 4 row-chunks per image
FREE = NT * W              # 2048
IMGS = B // N_CORES        # 4 images per core
SCALE = 1000.0 / 9.0       # folds the 1/9 box normalization into the sigmoid
BIAS1 = -0.01 * 1000.0     # sigmoid((s/9 - 0.01)*1000) = sigmoid(s*SCALE - 10)
BIAS2 = -0.9 * 1000.0

_BF16 = mybir.dt.bfloat16
_F32 = mybir.dt.float32


def _band_matrix() -> np.ndarray:
    """T[k, j] = 1 iff j in {k, k+1, k+2}; moving operand of every stage.

    rhs column j of contraction-chunk t maps to output position 128*t - 1 + j,
    so out gets taps from inputs 128*t+k with |out - in| <= 1.
    """
    t = np.zeros((P, 130), np.float32)
    k = np.arange(P)
    for d in range(3):
        t[k, k + d] = 1.0
    return t.astype(ml_dtypes.bfloat16)


def _bias_matrix() -> np.ndarray:
    """Per-partition bias columns for the two sigmoids (f32)."""
    b = np.empty((P, 2), np.float32)
    b[:, 0] = BIAS1
    b[:, 1] = BIAS2
    return b


def _emit_stage(nc, psum_ts, src, tb):
    """One separable 3-tap pass: contracts src's partition dim, transposed out.

    src:     SBUF bf16 [128, 2048], layout [d1-local-partition, (d1-chunk, d2)]
    psum_ts: list of PSUM f32 tiles jointly covering [128, 2048] in the layout
             [d2-local-partition, (d2-chunk, d1)] (1 tile of 4 banks or 2 of 2)
    """
    nts = len(psum_ts)
    per = NT // nts                # output chunks (banks) per psum tile
    for t in range(NT):            # contraction chunk (partition sections)
        j0 = 1 if t == 0 else 0
        j1 = 129 if t == NT - 1 else 130
        h0 = 128 * t - 1 + j0
        h1 = 128 * t - 1 + j1
        rhs = tb[:, j0:j1]
        for c in range(NT):        # output chunk (= PSUM bank)
            lhsT = src[:, t * W + 128 * c : t * W + 128 * c + 128]
            pt = psum_ts[c // per]
            out = pt[:, (c % per) * W + h0 : (c % per) * W + h1]
            nc.tensor.matmul(out, lhsT, rhs, start=(t == 0), stop=(t == NT - 1))


def _build_bass(reps: int = 1, split_psum: bool = False):
    nts = 2 if split_psum else 1   # psum tiles per stage
    psz = FREE // nts
    nc = bacc.Bacc("TRN2", target_bir_lowering=False, debug=False)
    x = nc.dram_tensor("x", [IMGS * H, W], _F32, kind="ExternalInput")
    tband = nc.dram_tensor("tband", [P, 130], _BF16, kind="ExternalInput")
    tbias = nc.dram_tensor("tbias", [P, 2], _F32, kind="ExternalInput")
    y = nc.dram_tensor("y", [IMGS * H, W], _F32, kind="ExternalOutput")

    with TileContext(nc) as tc:
        with (
            tc.tile_pool(name="const", bufs=1) as cpool,
            tc.tile_pool(name="xin", bufs=1) as xpool,
            tc.tile_pool(name="mid", bufs=2) as p1pool,
            tc.tile_pool(name="sig", bufs=4) as s1pool,
            tc.tile_pool(name="mid2", bufs=2) as p2pool,
            tc.tile_pool(name="outp", bufs=1) as opool,
            tc.tile_pool(name="psum", bufs=2 * nts, space="PSUM") as pspool,
        ):
            sig = mybir.ActivationFunctionType.Sigmoid

            for rep in range(reps):
                # Input loads are the very first instructions: SWDGE casts
                # f32->bf16 and streams all four images on one queue (the
                # aggregate SDMA/HBM rate is the cap — spreading across
                # HWDGE rings was measured slower).  Image 0 is split in
                # half so its first stage can start ~2us earlier.
                xts = []
                for i in range(IMGS):
                    xt = xpool.tile([P, FREE], _BF16, tag=f"x{i}", name=f"x_{i}")
                    halves = 2 if i == 0 else 1
                    step = NT // halves
                    for hh in range(halves):
                        nc.gpsimd.dma_start(
                            out=xt[:, hh * step * W : (hh + 1) * step * W].rearrange(
                                "p (t w) -> p t w", t=step
                            ),
                            in_=x[
                                (i * NT + hh * step) * P : (i * NT + (hh + 1) * step)
                                * P,
                                :,
                            ].rearrange("(t p) w -> p t w", p=P),
                        )
                    xts.append(xt)

                if rep == 0:
                    tb = cpool.tile([P, 130], _BF16)
                    nc.sync.dma_start(out=tb[:], in_=tband[:, :])
                    bias = cpool.tile([P, 2], _F32, tag="bias")
                    nc.sync.dma_start(out=bias[:], in_=tbias[:, :])
                    bias1, bias2 = bias[:, 0:1], bias[:, 1:2]

                    # HAM warm-up: ~28 matmuls on scratch data while the
                    # input DMAs stream.  Flips the PE clock gate to 8/8
                    # (2.4 GHz) before the first real stage; costs nothing
                    # (PE would be idle waiting on the loads anyway).
                    wsrc = cpool.tile([P, 256], _BF16, tag="wsrc")
                    nc.vector.memset(wsrc[:], 0.0)
                    wps = pspool.tile([P, psz], _F32, tag="ps", name="wps")
                    for _ in range(28):
                        nc.tensor.matmul(
                            wps[:, 0:256], wsrc[:, 0:128], wsrc[:, 0:256],
                            start=True, stop=True,
                        )

                # interleaved wave schedule: two image-pairs ping-pong the
                # two 4-bank PSUM slots; while one pair's PSUM stage drains
                # on DVE/ACT, the PE runs the other pair's matmuls.
                pa, p1, pb, s1, pc, p2, pd = ({} for _ in range(7))

                def stage(dst, src_map, i, nm):
                    dst[i] = [
                        pspool.tile([P, psz], _F32, tag="ps", name=f"{nm}{i}_{q}")
                        for q in range(nts)
                    ]
                    _emit_stage(nc, dst[i], src_map[i], tb)

                def copy(dst, src, i, pool, nm):
                    dst[i] = pool.tile([P, FREE], _BF16, tag=nm, name=f"{nm}{i}")
                    for q in range(nts):
                        nc.vector.tensor_copy(
                            dst[i][:, q * psz : (q + 1) * psz], src[i][q][:]
                        )

                def sig1(i):
                    s1[i] = s1pool.tile([P, FREE], _BF16, tag="s1", name=f"s1_{i}")
                    for q in range(nts):
                        nc.scalar.activation(
                            s1[i][:, q * psz : (q + 1) * psz],
                            pb[i][q][:],
                            sig,
                            bias=bias1,
                            scale=SCALE,
                        )

                def sig2(i):
                    # split halves: the store of the first half overlaps the
                    # sigmoid of the second (matters for the last image's tail)
                    ot = opool.tile([P, FREE], _F32, tag=f"o{i}", name=f"o_{i}")
                    hw = FREE // 2
                    rows_per_half = NT // 2 * P
                    for hh in range(2):
                        sl = slice(hh * hw, (hh + 1) * hw)
                        if nts == 2:
                            nc.scalar.activation(
                                ot[:, sl], pd[i][hh][:], sig, bias=bias2, scale=SCALE
                            )
                        else:
                            nc.scalar.activation(
                                ot[:, sl], pd[i][0][:, sl], sig, bias=bias2,
                                scale=SCALE,
                            )
                        nc.sync.dma_start(
                            out=y[
                                i * H + hh * rows_per_half : i * H
                                + (hh + 1) * rows_per_half,
                                :,
                            ].rearrange("(t p) w -> p t w", p=P),
                            in_=ot[:, sl].rearrange("p (t w) -> p t w", t=NT // 2),
                        )

                for i in (0, 1):
                    stage(pa, dict(enumerate(xts)), i, "pa")
                for i in (0, 1):
                    copy(p1, pa, i, p1pool, "p1_")
                for i in (0, 1):
                    stage(pb, p1, i, "pb")
                for i in (0, 1):
                    sig1(i)
                for i in (2, 3):
                    stage(pa, dict(enumerate(xts)), i, "pa")
                for i in (2, 3):
                    copy(p1, pa, i, p1pool, "p1_")
                for i in (2, 3):
                    stage(pb, p1, i, "pb")
                for i in (2, 3):
                    sig1(i)
                for i in (0, 1):
                    stage(pc, s1, i, "pc")
                for i in (0, 1):
                    copy(p2, pc, i, p2pool, "p2_")
                for i in (0, 1):
                    stage(pd, p2, i, "pd")
                for i in (0, 1):
                    sig2(i)
                for i in (2, 3):
                    stage(pc, s1, i, "pc")
                for i in (2, 3):
                    copy(p2, pc, i, p2pool, "p2_")
                for i in (2, 3):
                    stage(pd, p2, i, "pd")
                for i in (2, 3):
                    sig2(i)
    nc.compile()
    return nc


_NC_CACHE = {}


def _get_nc(reps: int = 1):
    if reps not in _NC_CACHE:
        _NC_CACHE[reps] = _build_bass(reps)
    return _NC_CACHE[reps]


def kernel_with_results(inputs: np.ndarray, **run_kwargs):
    """inputs: [32, 1, 512, 512] f32. Returns (out [32,1,512,512] f32, results)."""
    x = np.asarray(inputs)
    assert x.shape == (B, 1, H, W), x.shape
    x = np.ascontiguousarray(x.reshape(B, H, W), dtype=np.float32)
    tb = np.ascontiguousarray(_band_matrix())
    tbias = np.ascontiguousarray(_bias_matrix())

    in_maps = []
    for k in range(N_CORES):
        xk = np.ascontiguousarray(
            x[k * IMGS : (k + 1) * IMGS].reshape(IMGS * H, W)
        )
        in_maps.append({"x": xk, "tband": tb, "tbias": tbias})

    nc = _get_nc()
    res = run_bass_kernel_spmd(nc, in_maps, core_ids=list(range(N_CORES)), **run_kwargs)
    out = np.empty((B, H, W), dtype=np.float32)
    for k in range(N_CORES):
        out[k * IMGS : (k + 1) * IMGS] = (
            np.asarray(res.results[k]["y"]).astype(np.float32).reshape(IMGS, H, W)
        )
    return out.reshape(B, 1, H, W), res


def kernel(inputs: np.ndarray) -> np.ndarray:
    out, _ = kernel_with_results(inputs)
    return out


if __name__ == "__main__":
    rng = np.random.default_rng(0)
    demo = rng.random((B, 1, H, W), dtype=np.float32)
    out = kernel(demo)
    print("out", out.shape, out.dtype, float(out.min()), float(out.max()))

